# revision 3
# baseline (speedup 1.0000x reference)
"""Trainium2 Bass kernel for nn_CDER_64493228917301 (gnn_message_passing).

Reference semantics (GATConv-style, DGL u_dot_v / v_mul_e):
    el  = (e_ft @ W.T).reshape(N, H, F)
    e   = leaky_relu(einsum('ehf,ehf->eh', el[src], el[dst]))
    a   = segment_softmax(e, dst)          # softmax over edges sharing dst
    msg = ft[dst] * a[:, :, None]          # NOTE: uses DESTINATION features
    out = (segment_sum(msg, dst) + bias.reshape(1,H,F)).mean(axis=1)

Key algebraic identity: because the message uses ft[dst] (not ft[src]),
every edge in dst-segment n contributes ft[n] * a_e, and the softmax
weights a_e of one segment sum to 1.  Hence

    segment_sum(msg, dst)[n] = ft[n] * (1 if node n has >=1 in-edge else 0)

exactly (up to f32 rounding of order 1e-7 -- verified global rel err
1.2e-7 vs the jax reference).  The attention logits, the e_ft @ W matmul
and the edge gathers cancel out of the output entirely; the only thing
the edge list contributes is the per-node "has in-edge" indicator.

So the kernel computes, fully on device:

    out[n, f] = (sum_h ft[n, h, f]) * fscale[n] + bias_mean[f]

where fscale[n] = 0.25 * has_in_edge[n] (the 1/H fold is free) and
bias_mean = bias.reshape(H, F).mean(0).  The indicator is produced on
the host during input sharding (a single vectorized scatter over dst --
index preprocessing, like the sharding itself).

Distribution: node-parallel across the 8 NeuronCores.  Each core gets a
12500-node shard (padded to 12544 = 98*128) and streams its 6.4 MB of
ft through SBUF, which makes the kernel purely HBM-bandwidth-bound --
the target regime.

Implementation is raw Bass (no Tile framework): the Tile scheduler's
entry/exit drain + all-engine-barrier ceremony costs ~15 us on a ~25 us
kernel.  Manual pipeline:
  - SP (sync) HWDGE ring:   7x 896 KB ft tile loads, free-running
  - ACT (scalar) HWDGE ring: fscale/bias const loads + 7x 229 KB stores
    (separate ring so stores' sem-waits never block load issue)
  - DVE (vector): per tile: pairwise head-add, head-add, *fscale, +bias
  - GpSimd: end-of-kernel semaphore range-clear (so the NEFF can be
    re-executed) gated on per-engine done increments
"""

import numpy as np

N = 100000
H = 4
F = 32
D = H * F            # 128 floats per node in ft
NC = 8               # cores
PER = N // NC        # 12500 nodes per core
P = 128              # SBUF partitions
G = 14               # node-groups per partition per tile
B = 7                # tiles per core
PAD = P * G * B      # 12544 padded nodes per core
NBUF = 3             # ft / out buffer slots

_cached = None


def _build_bass():
    import concourse.bass as bass
    from concourse import mybir

    f32 = mybir.dt.float32
    nc = bass.Bass(
        "TRN2",
        target_bir_lowering=False,
        debug=False,
        num_devices=NC,
    )
    ft_in = nc.dram_tensor("ft_in", [PAD, D], f32, kind="ExternalInput").ap()
    fs_in = nc.dram_tensor("fs_in", [PAD], f32, kind="ExternalInput").ap()
    bias_in = nc.dram_tensor("bias_in", [P, F], f32, kind="ExternalInput").ap()
    out = nc.dram_tensor("out", [PAD, F], f32, kind="ExternalOutput").ap()

    # node index n (within the core's shard) = p*(G*B) + b*G + g
    ftv = ft_in.rearrange("(p b g) d -> b p (g d)", p=P, b=B, g=G)   # [B,128,G*D]
    fsv = fs_in.rearrange("(p x) -> p x", p=P)                        # [128, B*G]
    outv = out.rearrange("(p b g) f -> b p (g f)", p=P, b=B, g=G)     # [B,128,G*F]

    sem_ft = nc.alloc_semaphore("sem_ft")      # ft loads     (+16 each, SP ring)
    sem_cb = nc.alloc_semaphore("sem_cb")      # consts+stores (+16 each, ACT ring)
    sem_comp = nc.alloc_semaphore("sem_comp")  # per-tile compute done (+1)
    sem_done = nc.alloc_semaphore("sem_done")  # per-engine finished (+1)

    with (
        nc.sbuf_tensor("ft_buf", [P, NBUF * G * D], f32) as ft_buf,
        nc.sbuf_tensor("u_buf", [P, 2 * G * F], f32) as u_buf,
        nc.sbuf_tensor("o_buf", [P, NBUF * G * F], f32) as o_buf,
        nc.sbuf_tensor("fs_buf", [P, B * G], f32) as fs_buf,
        nc.sbuf_tensor("bias_buf", [P, F], f32) as bias_buf,
        nc.Block() as block,
    ):
        GD, GF = G * D, G * F

        @block.sync
        def _(sync):
            for b in range(B):
                ld = sync.dma_start(
                    ft_buf[:, (b % NBUF) * GD : (b % NBUF + 1) * GD], ftv[b]
                )
                if b >= NBUF:
                    # slot free once compute of tile b-NBUF fully done
                    ld._wait_ge(sem_comp, b - NBUF + 1)
                ld.then_inc(sem_ft, 16)
            sync.wait_ge(sem_ft, 16 * B).then_inc(sem_done, 1)

        @block.scalar
        def _(scalar):
            scalar.dma_start(fs_buf[:], fsv).then_inc(sem_cb, 16)
            scalar.dma_start(bias_buf[:], bias_in).then_inc(sem_cb, 16)
            for b in range(B):
                st = scalar.dma_start(
                    outv[b], o_buf[:, (b % NBUF) * GF : (b % NBUF + 1) * GF]
                )
                st._wait_ge(sem_comp, b + 1)
                st.then_inc(sem_cb, 16)
            scalar.wait_ge(sem_cb, 16 * (B + 2)).then_inc(sem_done, 1)

        @block.vector
        def _(vector):
            bias_bc = bias_buf[:].unsqueeze(1).broadcast_to([P, G, F])
            for b in range(B):
                ft_t = ft_buf[:, (b % NBUF) * GD : (b % NBUF + 1) * GD]
                o3 = (
                    o_buf[:, (b % NBUF) * GF : (b % NBUF + 1) * GF]
                    .rearrange("p (g f) -> p g f", f=F)
                )
                u3 = u_buf[:].rearrange("p (j g f) -> p j g f", j=2, g=G)
                in4 = ft_t.rearrange("p (g hh f) -> p g hh f", g=G, hh=H)
                # u[p,j,g,f] = ft[h=j] + ft[h=j+2]
                op1 = vector.tensor_add(
                    u3,
                    in4[:, :, 0:2, :].rearrange("p g j f -> p j g f"),
                    in4[:, :, 2:4, :].rearrange("p g j f -> p j g f"),
                )
                op1._wait_ge(sem_ft, 16 * (b + 1))
                # o = u0 + u1  (= sum over heads)
                op2 = vector.tensor_add(o3, u3[:, 0], u3[:, 1])
                if b >= NBUF:
                    # o slot free once store of tile b-NBUF retired
                    op2._wait_ge(sem_cb, 16 * (2 + b - NBUF + 1))
                # o *= fscale (per-node, broadcast over F)
                fs_bc = (
                    fs_buf[:, b * G : (b + 1) * G]
                    .unsqueeze(2)
                    .broadcast_to([P, G, F])
                )
                op3 = vector.tensor_mul(o3, o3, fs_bc)
                if b == 0:
                    # >=32: both const DMAs fully retired.  (>=16 is NOT
                    # "first DMA done" -- with two in flight, 16 slice-incs
                    # can arrive from 8 engines x 2 DMAs.)
                    op3._wait_ge(sem_cb, 32)
                # o += bias_mean (broadcast over partitions/groups)
                op4 = vector.tensor_add(o3, o3, bias_bc)
                if b == 0:
                    op4._wait_ge(sem_cb, 32)
                op4.then_inc(sem_comp, 1)
            vector.wait_ge(sem_comp, B).then_inc(sem_done, 1)

        @block.gpsimd
        def _(gpsimd):
            gpsimd.sem_clear(sem_ft)._wait_ge(sem_done, 3)
            gpsimd.sem_clear(sem_cb)
            gpsimd.sem_clear(sem_comp)
            gpsimd.sem_clear(sem_done)

    return nc


# results of the last device run (for test harness introspection)
LAST_RESULTS = None


def kernel(ft, e_ft, W, bias, src, dst):
    global _cached, LAST_RESULTS
    from concourse import bass_utils

    ft = np.ascontiguousarray(np.asarray(ft, dtype=np.float32)).reshape(N, D)
    bias = np.asarray(bias, dtype=np.float32)
    dst = np.asarray(dst)

    # per-node in-edge indicator, folded with the 1/H of the head mean
    fscale = np.zeros(N, np.float32)
    fscale[dst] = 1.0 / H
    bias_mean = bias.reshape(H, F).mean(axis=0)
    bias_b = np.ascontiguousarray(np.broadcast_to(bias_mean, (P, F)))

    in_maps = []
    for c in range(NC):
        ft_s = np.zeros((PAD, D), np.float32)
        ft_s[:PER] = ft[c * PER : (c + 1) * PER]
        fs_s = np.zeros(PAD, np.float32)
        fs_s[:PER] = fscale[c * PER : (c + 1) * PER]
        in_maps.append({"ft_in": ft_s, "fs_in": fs_s, "bias_in": bias_b})

    if _cached is None:
        _cached = _build_bass()
    nc = _cached

    res = bass_utils.run_bass_kernel_spmd(nc, in_maps, core_ids=list(range(NC)))
    LAST_RESULTS = res
    out = np.empty((N, F), np.float32)
    for c in range(NC):
        out[c * PER : (c + 1) * PER] = res.results[c]["out"][:PER]
    return out
